# revision 1
# baseline (speedup 1.0000x reference)
"""GRU greedy decoder on 8 trn2 cores.

Vocab-sharded: each core owns 4000 vocab entries of the out-projection; per
step it computes its local (B=32, 4000) logits, finds the local argmax, all
cores exchange (max-value, global-index) candidates, everyone computes the
global argmax, gathers the fused embedding row G'[tok] = emb[tok]@W_ih.T+b_ih,
and advances the replicated GRU state. Logits are staged to SBUF and DMA'd to
each core's output stripe.

Layout:
  PSUM logits tile [128, 1024]: partition 32j + m (j = psum col group, m =
  batch), free = pos in [0, 1024); vocab v = core*4000 + j*1000 + pos for
  pos < 1000; pos in [1000, 1024) are pad slots with bias -1e30.
  Matmuls: col-group tiling only (row groups != 0 crash at runtime on this
  stack): per j, two N=512 matmuls; lhsT = hT_aug [17, 32] at partitions 0:17.
"""

import numpy as np
import concourse.bass as bass
import concourse.bacc as bacc
import concourse.mybir as mybir
from concourse import tile
from concourse.bass import AP, IndirectOffsetOnAxis
from concourse.tile_rust import add_dep_helper

FP = mybir.dt.float32
B, H, E, V, T = 32, 16, 16, 32000, 100
NCORES = 8
VLOC = V // NCORES           # 4000
F = VLOC // 4                # 1000 valid entries per partition
FPAD = 1024                  # padded free size (PSUM bank alignment)
K = H + 1                    # 17: h dims + ones row (bias)
G3 = 3 * H                   # 48


def build_nc(n_steps=T, exchange="allgather", psum_max=True, reps=1):
    nc = bacc.Bacc(None, target_bir_lowering=False)

    # ---- I/O ----
    wt_in = nc.dram_tensor("wt", [K, 4 * FPAD], FP, kind="ExternalInput")
    whh_in = nc.dram_tensor("whh", [K, G3], FP, kind="ExternalInput")
    h0t_in = nc.dram_tensor("h0t", [K, B], FP, kind="ExternalInput")
    h0a_in = nc.dram_tensor("h0a", [B, K], FP, kind="ExternalInput")
    off2_in = nc.dram_tensor("off2", [128, 1], FP, kind="ExternalInput")
    ident_in = nc.dram_tensor("ident", [128, 128], FP, kind="ExternalInput")
    gp_in = nc.dram_tensor("gp", [V, G3], FP, kind="ExternalInput")
    out_dram = nc.dram_tensor("out", [B, n_steps, VLOC], FP, kind="ExternalOutput")

    with tile.TileContext(nc) as tc:
        with (
            tc.tile_pool(name="const", bufs=1) as cpool,
            tc.tile_pool(name="state", bufs=1) as spool,
            tc.tile_pool(name="work", bufs=4) as wpool,
            tc.tile_pool(name="stage", bufs=3) as stpool,
            tc.tile_pool(name="psumL", bufs=2, space="PSUM") as plpool,
            tc.tile_pool(name="psumG", bufs=1, space="PSUM") as pgpool,
            tc.tile_pool(name="psumT", bufs=1, space="PSUM") as ptpool,
            tc.tile_pool(name="dram", bufs=4, space="DRAM") as dpool,
        ):
            # ---- constants to SBUF ----
            wt_sb = cpool.tile([K, 4 * FPAD], FP, tag="wt")
            nc.gpsimd.dma_start(out=wt_sb[:, :], in_=wt_in[:, :])
            whh_sb = cpool.tile([K, G3], FP, tag="whh")
            nc.gpsimd.dma_start(out=whh_sb[:, :], in_=whh_in[:, :])
            off2_sb = cpool.tile([128, 1], FP, tag="off2")
            nc.gpsimd.dma_start(out=off2_sb[:, :], in_=off2_in[:, :])
            ident_sb = cpool.tile([128, 128], FP, tag="ident")
            nc.gpsimd.dma_start(out=ident_sb[:, :], in_=ident_in[:, :])

            # ---- state ----
            hT_sb = spool.tile([K, B], FP, tag="hT")     # hT_aug [17, 32]
            nc.gpsimd.dma_start(out=hT_sb[:, :], in_=h0t_in[:, :])
            ha_sb = spool.tile([B, K], FP, tag="ha")     # h_aug [32, 17] col 16 = ones
            nc.gpsimd.dma_start(out=ha_sb[:, :], in_=h0a_in[:, :])
            vs32 = spool.tile([B, B], FP, tag="vs32")    # row 0 = selected idx
            nc.vector.memset(vs32[:, :], 0.0)

            RING = 4
            patches = []   # (BassInstruction, sem, threshold) applied post-schedule
            if exchange == "remote":
                rsem = nc.alloc_semaphore("rsem")
                lsem = nc.alloc_semaphore("lsem")
                rcvb = spool.tile([128, RING * 16], FP, tag="rcvb")
                nc.vector.memset(rcvb[:, :], 0.0)
                pid = nc.gpsimd.partition_id()
                poff = pid * 2

            for rep in range(reps):
              for t in range(n_steps):
                # ---------- logits matmuls (col-group tiling) ----------
                lps = plpool.tile([128, FPAD], FP, tag="L")
                for j in range(4):
                    for hh in range(2):
                        nc.tensor.matmul(
                            out=lps[32 * j:32 * j + B, hh * 512:(hh + 1) * 512],
                            lhsT=hT_sb[:, :],
                            rhs=wt_sb[:, j * FPAD + hh * 512:j * FPAD + (hh + 1) * 512],
                            start=True, stop=True,
                            tile_position=(0, 32 * j),
                        )
                # gh = h @ W_hh.T + b_hh  -> [32, 48]
                ghp = pgpool.tile([B, G3], FP, tag="gh")
                nc.tensor.matmul(
                    out=ghp[:, :], lhsT=hT_sb[:, :], rhs=whh_sb[:, :],
                    start=True, stop=True, tile_position=(0, 0),
                )

                # ---------- stage to SBUF for output (4-step slabs) ----------
                if t % 4 == 0:
                    slab = stpool.tile([128, 4 * FPAD], FP, tag="stg4")
                stg = slab[:, (t % 4) * FPAD:(t % 4 + 1) * FPAD]
                nc.scalar.copy(out=stg, in_=lps[:, :])

                # ---------- local argmax ----------
                mx8 = wpool.tile([128, 8], FP, tag="mx8")
                mi8 = wpool.tile([128, 8], mybir.dt.uint32, tag="mi8")
                src = lps
                nc.vector.max(out=mx8[:, :], in_=src[:, :])
                nc.vector.max_index(out=mi8[:, :], in_max=mx8[:, :], in_values=src[:, :])

                # payload CW [128, 2] = (val, global idx as fp32)
                cw = wpool.tile([128, 2], FP, tag="cw")
                if exchange == "remote" and t >= 3:
                    wn = nc.vector.nop(nofuse=True, hint="lsem_wait")
                    patches.append((wn, lsem, 16 * (t - 2)))
                cwi1 = nc.vector.tensor_copy(out=cw[:, 0:1], in_=mx8[:, 0:1])
                cwi2 = nc.vector.tensor_scalar_add(cw[:, 1:2], mi8[:, 0:1], off2_sb[:, 0:1])
                if exchange == "remote" and t >= 3:
                    add_dep_helper(wn.ins, cwi1.ins, sync=False, reason="lsem order")
                    add_dep_helper(wn.ins, cwi2.ins, sync=False, reason="lsem order")

                # ---------- exchange ----------
                if exchange.startswith("allgather"):
                    cc_in = dpool.tile([128, 2], FP, tag="ccin")
                    cc_out = dpool.tile([128 * NCORES, 2], FP, tag="ccout")
                    nc.scalar.dma_start(out=cc_in[:, :], in_=cw[:, :])
                    nc.gpsimd.collective_compute(
                        "AllGather",
                        mybir.AluOpType.bypass,
                        ins=[cc_in[:, :].opt()],
                        outs=[cc_out[:, :].opt()],
                        replica_groups=[list(range(NCORES))],
                    )
                    rcv = wpool.tile([128, NCORES * 2], FP, tag="rcv")
                    # iterate (p, c, x): strides in elements: p->2, c->256, x->1
                    nc.scalar.dma_start(
                        out=rcv[:, :],
                        in_=AP(cc_out[:, :].tensor, 0,
                               [[2, 128], [128 * 2, NCORES], [1, 2]]),
                    )
                elif exchange == "remote":
                    slot = t % RING
                    out_ap = AP(rcvb[:, :].tensor, slot * 16 + poff,
                                [[RING * 16, 128], [1, 2]])
                    nc.gpsimd.remote_dma_broadcast(
                        out_ap=out_ap, in_ap=cw[:, :],
                        remote_sem=rsem, local_sem=lsem,
                        rdests=[(0, k) for k in range(NCORES)],
                    )
                    nc.gpsimd.trigger_dma(count=None)
                    rcv = rcvb[:, slot * 16:(slot + 1) * 16]
                elif exchange == "none":
                    rcv = None
                else:
                    raise ValueError(exchange)

                cw2 = wpool.tile([128, 2], FP, tag="cw2")
                if rcv is not None and exchange in ("allgather", "remote"):
                    # core-combine: best over 8 cores per (j, m) partition
                    rm8 = wpool.tile([128, 8], FP, tag="rm8")
                    if exchange == "remote":
                        rr = rcv.rearrange("p (c x) -> p c x", x=2)
                    else:
                        rr = rcv[:, :].rearrange("p (c x) -> p c x", x=2)
                    vals = rr[:, :, 0]
                    idxs = rr[:, :, 1]
                    if exchange == "remote":
                        wr = nc.vector.nop(nofuse=True, hint="rsem_wait")
                        patches.append((wr, rsem, 16 * (t + 1)))
                    mxi = nc.vector.max(out=rm8[:, :], in_=vals)
                    if exchange == "remote":
                        add_dep_helper(wr.ins, mxi.ins, sync=False, reason="rsem order")
                    mskc = wpool.tile([128, 8], FP, tag="mskc")
                    nc.vector.tensor_scalar(
                        out=mskc[:, :], in0=vals, scalar1=rm8[:, 0:1], scalar2=None,
                        op0=mybir.AluOpType.is_equal,
                    )
                    tmpc = wpool.tile([128, 8], FP, tag="tmpc")
                    nc.vector.tensor_tensor(out=tmpc[:, :], in0=mskc[:, :], in1=idxs,
                                            op=mybir.AluOpType.mult)
                    gidxc = wpool.tile([128, 1], FP, tag="gidxc")
                    nc.vector.tensor_reduce(
                        out=gidxc[:, :], in_=tmpc[:, :], axis=mybir.AxisListType.X,
                        op=mybir.AluOpType.add,
                    )
                    cwv, cwi = rm8[:, 0:1], gidxc[:, :]
                else:
                    if rcv is not None:
                        nc.vector.tensor_copy(out=cw2[:, :], in_=rcv[:, 0:2])
                    else:
                        nc.vector.tensor_copy(out=cw2[:, :], in_=cw[:, :])
                    cwv, cwi = cw2[:, 0:1], cw2[:, 1:2]

                # ---------- j-combine ----------
                tj = ptpool.tile([1, 256], FP, tag="tj")
                tjv = tj[:, 0:128]
                tji = tj[:, 128:256]
                nc.tensor.transpose(out=tjv, in_=cwv, identity=ident_sb[:, :])
                nc.tensor.transpose(out=tji, in_=cwi, identity=ident_sb[:, :])
                gmj = wpool.tile([1, B], FP, tag="gmj")
                vrow = tjv.rearrange("p (j m) -> p m j", j=4)
                irow = tji.rearrange("p (j m) -> p m j", j=4)
                nc.vector.tensor_reduce(
                    out=gmj[:, :], in_=vrow, axis=mybir.AxisListType.X,
                    op=mybir.AluOpType.max,
                )
                msk2 = wpool.tile([1, 128], FP, tag="msk2")
                m2v = msk2[:, :].rearrange("p (j m) -> p m j", j=4)
                nc.vector.tensor_tensor(
                    out=m2v, in0=vrow,
                    in1=gmj[:, :].unsqueeze(2).to_broadcast([1, B, 4]),
                    op=mybir.AluOpType.is_equal,
                )
                tmp2 = wpool.tile([1, 128], FP, tag="tmp2")
                t2v = tmp2[:, :].rearrange("p (j m) -> p m j", j=4)
                nc.vector.tensor_tensor(out=t2v, in0=m2v, in1=irow,
                                        op=mybir.AluOpType.mult)
                nc.vector.tensor_reduce(
                    out=vs32[0:1, 0:B], in_=t2v, axis=mybir.AxisListType.X,
                    op=mybir.AluOpType.add,
                )
                # transpose row -> column, cast to int
                vs32t = wpool.tile([B, B], FP, tag="vs32t")
                nc.vector.transpose(out=vs32t[:, :], in_=vs32[:, :])
                idxi = wpool.tile([B, 1], mybir.dt.int32, tag="idxi")
                nc.vector.tensor_copy(out=idxi[:, :], in_=vs32t[:, 0:1])

                # ---------- gather G'[tok] ----------
                xg = wpool.tile([B, G3], FP, tag="xg")
                nc.gpsimd.indirect_dma_start(
                    out=xg[:, :], out_offset=None,
                    in_=gp_in[:, :],
                    in_offset=IndirectOffsetOnAxis(ap=idxi[:, 0:1], axis=0),
                )

                # ---------- GRU ----------
                rzp = wpool.tile([B, 2 * H], FP, tag="rzp")
                nc.vector.tensor_add(out=rzp[:, :], in0=xg[:, 0:2 * H], in1=ghp[:, 0:2 * H])
                rz = wpool.tile([B, 2 * H], FP, tag="rz")
                nc.scalar.activation(out=rz[:, :], in_=rzp[:, :],
                                     func=mybir.ActivationFunctionType.Sigmoid)
                rh = wpool.tile([B, H], FP, tag="rh")
                nc.vector.tensor_mul(out=rh[:, :], in0=rz[:, 0:H], in1=ghp[:, 2 * H:G3])
                npre = wpool.tile([B, H], FP, tag="npre")
                nc.vector.tensor_add(out=npre[:, :], in0=xg[:, 2 * H:G3], in1=rh[:, :])
                nn_ = wpool.tile([B, H], FP, tag="nn")
                nc.scalar.activation(out=nn_[:, :], in_=npre[:, :],
                                     func=mybir.ActivationFunctionType.Tanh)
                dd = wpool.tile([B, H], FP, tag="dd")
                nc.vector.tensor_sub(out=dd[:, :], in0=ha_sb[:, 0:H], in1=nn_[:, :])
                zd = wpool.tile([B, H], FP, tag="zd")
                nc.vector.tensor_mul(out=zd[:, :], in0=rz[:, H:2 * H], in1=dd[:, :])
                nc.vector.tensor_add(out=ha_sb[:, 0:H], in0=nn_[:, :], in1=zd[:, :])

                # hT update: transpose ha [32, 17] -> [17, 32]
                htt = ptpool.tile([K, B], FP, tag="htt")
                nc.tensor.transpose(out=htt[:, :], in_=ha_sb[:, :],
                                    identity=ident_sb[0:B, 0:B])
                nc.vector.tensor_copy(out=hT_sb[:, :], in_=htt[:, :])

                # ---------- output DMA ----------
                if t % 4 == 3 or t == n_steps - 1:
                    nt = t % 4 + 1
                    t0_ = t - nt + 1
                    for tt in range(nt):
                        dst = AP(out_dram, (t0_ + tt) * VLOC,
                                 [[1000, 4], [n_steps * VLOC, B], [1, F]])
                        nc.scalar.dma_start(
                            out=dst, in_=slab[:, tt * FPAD:tt * FPAD + F])

    for bi, sem, thr in patches:
        bi.wait_op(sem, thr, "sem-ge")
    nc.finalize()
    return nc


def host_prep(inputs, n_steps=T):
    """Build per-core input maps from the full problem inputs."""
    emb = np.asarray(inputs["embedding"], np.float32)
    W_ih = np.asarray(inputs["W_ih"], np.float32)
    W_hh = np.asarray(inputs["W_hh"], np.float32)
    b_ih = np.asarray(inputs["b_ih"], np.float32)
    b_hh = np.asarray(inputs["b_hh"], np.float32)
    W_out = np.asarray(inputs["W_out"], np.float32)
    b_out = np.asarray(inputs["b_out"], np.float32)
    h0 = np.asarray(inputs["encoder_hidden"], np.float32)[0]  # [B, H]

    # G' = emb @ W_ih.T + b_ih  [V, 48]
    gp = (emb @ W_ih.T + b_ih).astype(np.float32)
    # W_aug [V, 17]
    w_aug = np.concatenate([W_out, b_out[:, None]], axis=1).astype(np.float32)
    # whh_aug.T [17, 48]
    whh = np.concatenate([W_hh.T, b_hh[None, :]], axis=0).astype(np.float32)

    # The kernel's iteration t computes logits_t from its current state, so the
    # initial state must be h1 = GRU(emb[SOS=0], h0), computed here in fp32.
    x0 = np.broadcast_to(emb[0], (B, E))
    gi = (x0 @ W_ih.T + b_ih).astype(np.float32)
    gh = (h0 @ W_hh.T + b_hh).astype(np.float32)
    i_r, i_z, i_n = gi[:, :H], gi[:, H:2 * H], gi[:, 2 * H:]
    h_r, h_z, h_n = gh[:, :H], gh[:, H:2 * H], gh[:, 2 * H:]
    r = (1.0 / (1.0 + np.exp(-(i_r + h_r), dtype=np.float32))).astype(np.float32)
    z = (1.0 / (1.0 + np.exp(-(i_z + h_z), dtype=np.float32))).astype(np.float32)
    n = np.tanh(i_n + r * h_n, dtype=np.float32).astype(np.float32)
    h1 = ((1.0 - z) * n + z * h0).astype(np.float32)

    h0a = np.concatenate([h1, np.ones((B, 1), np.float32)], axis=1)  # [32, 17]
    h0t = h0a.T.copy()                                               # [17, 32]

    ident = np.eye(128, dtype=np.float32)

    in_maps = []
    for c in range(NCORES):
        wt = np.zeros((K, 4 * FPAD), np.float32)
        for j in range(4):
            blk = np.zeros((K, FPAD), np.float32)
            blk[K - 1, :] = -1.0e30          # pad slots: bias -inf
            v0 = c * VLOC + j * 1000
            blk[:, 0:F] = w_aug[v0:v0 + F, :].T
            wt[:, j * FPAD:(j + 1) * FPAD] = blk
        off2 = np.zeros((128, 1), np.float32)
        for j in range(4):
            off2[32 * j:32 * j + 32, 0] = c * VLOC + j * 1000
        in_maps.append({
            "wt": wt, "whh": whh, "h0t": h0t, "h0a": h0a,
            "off2": off2, "ident": ident, "gp": gp,
        })
    return in_maps


def assemble_output(results, n_steps=T):
    """Concatenate per-core [B, T, VLOC] stripes into [B, T, V]."""
    return np.concatenate([r["out"] for r in results], axis=2)


_NC_CACHE = {}


def kernel(**inputs):
    """Full-input entrypoint: shard across 8 NeuronCores, run the Bass kernel,
    return the full (32, 100, 32000) float32 logits tensor."""
    from concourse.bass_utils import run_bass_kernel_spmd

    key = ("allgather", T)
    if key not in _NC_CACHE:
        _NC_CACHE[key] = build_nc(n_steps=T, exchange="allgather")
    nc = _NC_CACHE[key]
    in_maps = host_prep(inputs)
    res = run_bass_kernel_spmd(nc, in_maps, core_ids=list(range(NCORES)))
    return assemble_output(res.results)

